# revision 6
# baseline (speedup 1.0000x reference)
"""CPC loss kernel for Trainium2 (Bass/Tile), data-parallel over batch on 8 NeuronCores.

Math: the reference's exp/log cancel exactly, so the loss is a masked sum of
dot products:
    loss = -(1/(K*B*(T-1))) * sum_{b,e,k,t} a_k * mctx[b,t,e,k] * mask[b,t]
                                * (base[b,t+k+1,e] - negsum[b,e])
with a_k = (T-1)/(T-1-k) folding the per-step 1/(T-i) normalization and
negsum[b] = sum_n base.reshape(B*T,E)[neg_ids[b,n]].

Because the loss is linear in mapped_ctx, the k-reduction is a "diagonal sum"
over shifted planes: with mm_k[e,t] = a_k*mask*mctx[...,k] (host-prepped fp8,
pre-shifted by k+1 and zero-padded inside each plane's T-wide window),
    g[e,s] = sum_k mm_k[e, s]            (shifted-plane accumulation)
    S[r]   = sum_{e,s} g[e,s] * bmn[e,s]
bmn's zero tail (s >= T never occurs: the shifted planes drop their last k+1
masked columns, which is exactly the reference's t < T-i trimming).

Device pipeline, per core (8 batch rows):
  - One plain HWDGE DMA per row brings all 8 pre-shifted fp8 planes (1 MB).
  - PE accumulates g in PSUM via identity-stationary matmuls: out[e,s] +=
    rhs[e,s] with rhs = plane k's window. fp8e4 DoubleRow mode processes two
    planes per instruction at 2 cols/cycle, so a row costs ~8 matmuls of 512
    cols. The stationary is the identity loaded twice ([128, 2, 128]).
  - DVE: prod = ps * bmn (PSUM x SBUF -> fp16), reduce_sum -> fp32 partials
    [E, 1] per row; partials DMA out; host does the final 128-sums + scale.

Per-core HBM traffic: 8 MB fp8 planes + 2 MB fp16 bmn (vs 32 MB fp32 input
share): ~28 us at the 358 GB/s HBM/NC limit. PE ~20 us, DVE ~20 us, all
overlapped behind DMA.

fp8(e4m3) quantization of mapped_ctx costs ~1.1e-2 relative error on the
final scalar (gate: 2e-2); mapped_ctx is pre-scaled by 16 (power of two,
exact) to stay in e4m3's normal range, compensated exactly in bmn (/16).
MODE "pacc16" is the same pipeline in fp16 (exact, ~2x traffic, plain
matmuls).
"""

import numpy as np

B, T, E, K, NNEG = 64, 1024, 128, 8, 64
NCORES = 8
B_LOC = B // NCORES          # batch rows per core
SCALE = 16.0                 # power-of-2 pre-scale of mctx before fp8 cast
CH = 512                     # matmul chunk (one PSUM bank)

MODE = "pacc8"               # "pacc8" | "pacc16"
_CACHE = {}
TRACE = False                # test harness may flip this for NTFF profiling
TRACE_KWARGS = {}
LAST_RESULTS = None


def _build(mode):
    from contextlib import ExitStack
    import concourse.bass as bass
    import concourse.bacc as bacc
    import concourse.tile as tile
    import concourse.mybir as mybir

    f32 = mybir.dt.float32
    f16 = mybir.dt.float16
    fp8 = mode == "pacc8"
    in_dt = mybir.dt.float8e4 if fp8 else f16

    nc = bacc.Bacc(
        "TRN2",
        target_bir_lowering=False,
        debug=False,
        enable_asserts=False,
        num_devices=NCORES,
    )
    m_in = nc.dram_tensor("m8", [B_LOC, E, K, T], in_dt, kind="ExternalInput").ap()
    bmn_in = nc.dram_tensor("bmn", [B_LOC, E, T], f16, kind="ExternalInput").ap()
    id_in = nc.dram_tensor("ident", [E, 2 * E], in_dt, kind="ExternalInput").ap()
    p_out = nc.dram_tensor("P", [E, B_LOC], f32, kind="ExternalOutput").ap()

    with tile.TileContext(nc) as tc, ExitStack() as ctx:
        m_pool = ctx.enter_context(tc.tile_pool(name="m", bufs=1))
        b_pool = ctx.enter_context(tc.tile_pool(name="b", bufs=1))
        prod_pool = ctx.enter_context(tc.tile_pool(name="prod", bufs=4))
        misc_pool = ctx.enter_context(tc.tile_pool(name="misc", bufs=1))
        psum_pool = ctx.enter_context(tc.tile_pool(name="ps", bufs=4, space="PSUM"))

        ident = misc_pool.tile([E, 2 * E], in_dt)
        nc.scalar.dma_start(ident[:], id_in[:, :])
        part = misc_pool.tile([E, B_LOC], f32)

        # All input DMAs issued up front, rows alternating between the two
        # HWDGE queues so two transfers stream concurrently.
        mts, bts = [], []
        for r in range(B_LOC):
            q = nc.sync if r % 2 == 0 else nc.scalar
            mt = m_pool.tile([E, K, T], in_dt, tag=f"m{r}")
            q.dma_start(mt[:], m_in[r])
            mts.append(mt)
        for r in range(B_LOC):
            q = nc.sync if r % 2 == 1 else nc.scalar
            bt = b_pool.tile([E, T], f16, tag=f"bmn{r}")
            q.dma_start(bt[:], bmn_in[r])
            bts.append(bt)

        for r in range(B_LOC):
            mt = mts[r]
            ps = psum_pool.tile([E, T], f32, tag="ps")
            for c in range(T // CH):
                if fp8:
                    lhsT = bass.AP(ident[:].tensor, 0, [[2 * E, E], [E, 2], [1, E]])
                    for kp in range(K // 2):
                        rhs = bass.AP(
                            mt[:].tensor, 2 * kp * T + c * CH,
                            [[K * T, E], [T, 2], [1, CH]],
                        )
                        nc.tensor.matmul(
                            ps[:, c * CH:(c + 1) * CH], lhsT=lhsT, rhs=rhs,
                            start=(kp == 0), stop=(kp == K // 2 - 1),
                            perf_mode=mybir.MatmulPerfMode.DoubleRow,
                        )
                else:
                    lhsT = bass.AP(ident[:].tensor, 0, [[2 * E, E], [1, E]])
                    for k in range(K):
                        rhs = bass.AP(
                            mt[:].tensor, k * T + c * CH,
                            [[K * T, E], [1, CH]],
                        )
                        nc.tensor.matmul(
                            ps[:, c * CH:(c + 1) * CH], lhsT=lhsT, rhs=rhs,
                            start=(k == 0), stop=(k == K - 1),
                        )
            # Fused multiply+reduce on DVE: prod = ps * bmn (written fp16,
            # never re-read), accum_out = sum_s -> partials column.
            prod = prod_pool.tile([E, T], f16, tag="prod")
            nc.vector.scalar_tensor_tensor(
                prod[:], ps[:], 1.0, bts[r][:],
                op0=mybir.AluOpType.mult, op1=mybir.AluOpType.mult,
                accum_out=part[:, r:r + 1],
            )
        nc.scalar.dma_start(p_out[:, :], part[:])

    nc.compile()
    return nc


def kernel(base_emb, mapped_ctx, seq_lens, neg_ids):
    global LAST_RESULTS
    import ml_dtypes
    from concourse import bass_utils

    base = np.ascontiguousarray(np.asarray(base_emb, dtype=np.float32))
    mctx = np.asarray(mapped_ctx, dtype=np.float32)
    seq = np.asarray(seq_lens, dtype=np.int32)
    nids = np.asarray(neg_ids, dtype=np.int32)

    in_np_dt = ml_dtypes.float8_e4m3 if MODE == "pacc8" else np.float16

    # Host prep (sharding + per-batch-element negative gather per the
    # sharding hint; mask and per-step normalization fold into the linear
    # prefactors of mapped_ctx / base).
    neg_sum = base.reshape(B * T, E)[nids].sum(axis=1)             # [B, E]
    bmn = np.ascontiguousarray(
        ((base - neg_sum[:, None, :]) / SCALE).transpose(0, 2, 1)
        .astype(np.float16))                                       # [B, E, T]

    alpha = (SCALE * (T - 1.0) / (T - 1.0 - np.arange(K))).astype(np.float32)
    mask = (np.arange(T)[None, :] < seq[:, None]).astype(np.float32)  # [B, T]
    m_base = ((mctx.transpose(0, 2, 3, 1)                          # [B,E,K,T]
               * alpha[None, None, :, None]
               * mask[:, None, None, :]).astype(in_np_dt))
    mm = np.zeros((B, E, K, T), in_np_dt)
    for k in range(K):                                             # shift by k+1
        mm[:, :, k, k + 1:] = m_base[:, :, k, :T - 1 - k]

    ident = np.zeros((E, 2 * E), in_np_dt)
    ident[:, :E][np.arange(E), np.arange(E)] = 1.0
    ident[:, E:][np.arange(E), np.arange(E)] = 1.0

    key = ("nc", MODE)
    if key not in _CACHE:
        _CACHE[key] = _build(MODE)
    nc = _CACHE[key]

    in_maps = []
    for c in range(NCORES):
        sl = slice(c * B_LOC, (c + 1) * B_LOC)
        in_maps.append({
            "m8": mm[sl],
            "bmn": np.ascontiguousarray(bmn[sl]),
            "ident": ident,
        })

    res = bass_utils.run_bass_kernel_spmd(
        nc, in_maps, core_ids=list(range(NCORES)), trace=TRACE, **TRACE_KWARGS
    )
    LAST_RESULTS = res

    s_total = sum(float(r["P"].sum(dtype=np.float64)) for r in res.results)
    loss = -s_total / (K * B * (T - 1.0))
    return np.float32(loss)


# revision 7
# speedup vs baseline: 1.0405x; 1.0405x over previous
"""CPC loss kernel for Trainium2 (Bass/Tile), data-parallel over batch on 8 NeuronCores.

Math: the reference's exp/log cancel exactly, so the loss is a masked sum of
dot products:
    loss = -(1/(K*B*(T-1))) * sum_{b,e,k,t} a_k * mctx[b,t,e,k] * mask[b,t]
                                * (base[b,t+k+1,e] - negsum[b,e])
with a_k = (T-1)/(T-1-k) folding the per-step 1/(T-i) normalization and
negsum[b] = sum_n base.reshape(B*T,E)[neg_ids[b,n]].

Because the loss is linear in mapped_ctx, the k-reduction is a "diagonal sum"
over shifted planes: with mm_k[e,t] = a_k*mask*mctx[...,k] (host-prepped fp8,
pre-shifted by k+1 and zero-padded inside each plane's T-wide window),
    g[e,s] = sum_k mm_k[e, s]            (shifted-plane accumulation)
    S[r]   = sum_{e,s} g[e,s] * bmn[e,s]
(the shifted planes drop their last k+1 masked columns, which is exactly the
reference's t < T-i trimming).

Device pipeline, per core (8 batch rows):
  - One plain HWDGE DMA per row brings all 8 pre-shifted fp8 planes (<=1 MB).
  - PE accumulates g in PSUM via identity-stationary matmuls: fp8e4 DoubleRow
    processes two planes per instruction at 2 cols/cycle; the stationary is
    the identity loaded twice ([128, 2, 128]).
  - One fused DVE scalar_tensor_tensor per row: prod = ps * bmn (fp16,
    never re-read) with accum_out = fp32 partials [E, 1]. Partials DMA out;
    host does the final 128-sums + scale.

Rows are trimmed to their sequence length: all of a row's loss terms live at
s < seq+K+1, so DMA/matmul/STT widths shrink to SL = ceil128(min(seq+K+1,T)).
Rows are sorted by SL and snake-assigned to cores so the 8 per-slot static
widths (compiled into the NEFF, cached per distinct seq pattern) are balanced.

Per-core HBM traffic: ~7 MB fp8 planes + ~1.8 MB fp16 bmn (vs 32 MB fp32
input share). fp8(e4m3) quantization of mapped_ctx costs ~1.3e-2 relative
error on the final scalar (gate: 2e-2); mapped_ctx is pre-scaled by 16
(power of two, exact), compensated exactly in bmn (/16).
MODE "pacc16" is the same pipeline in fp16 (exact, ~2x traffic).
"""

import numpy as np

B, T, E, K, NNEG = 64, 1024, 128, 8, 64
NCORES = 8
B_LOC = B // NCORES          # batch rows per core
SCALE = 16.0                 # power-of-2 pre-scale of mctx before fp8 cast
CH = 512                     # matmul chunk (one PSUM bank)

MODE = "pacc8"               # "pacc8" | "pacc16"
TRIM = True                  # trim per-row widths to seq_len (recompiles per
                             # distinct seq pattern; NEFF cache makes it warm)
_CACHE = {}
TRACE = False                # test harness may flip this for NTFF profiling
TRACE_KWARGS = {}
LAST_RESULTS = None


def _build(mode, slots):
    from contextlib import ExitStack
    import concourse.bass as bass
    import concourse.bacc as bacc
    import concourse.tile as tile
    import concourse.mybir as mybir

    f32 = mybir.dt.float32
    f16 = mybir.dt.float16
    fp8 = mode == "pacc8"
    in_dt = mybir.dt.float8e4 if fp8 else f16

    nc = bacc.Bacc(
        "TRN2",
        target_bir_lowering=False,
        debug=False,
        enable_asserts=False,
        num_devices=NCORES,
    )
    m_in = nc.dram_tensor("m8", [B_LOC, E, K, T], in_dt, kind="ExternalInput").ap()
    bmn_in = nc.dram_tensor("bmn", [B_LOC, E, T], f16, kind="ExternalInput").ap()
    id_in = nc.dram_tensor("ident", [E, 2 * E], in_dt, kind="ExternalInput").ap()
    p_out = nc.dram_tensor("P", [E, B_LOC], f32, kind="ExternalOutput").ap()

    with tile.TileContext(nc) as tc, ExitStack() as ctx:
        m_pool = ctx.enter_context(tc.tile_pool(name="m", bufs=1))
        b_pool = ctx.enter_context(tc.tile_pool(name="b", bufs=1))
        prod_pool = ctx.enter_context(tc.tile_pool(name="prod", bufs=4))
        misc_pool = ctx.enter_context(tc.tile_pool(name="misc", bufs=1))
        psum_pool = ctx.enter_context(tc.tile_pool(name="ps", bufs=4, space="PSUM"))

        ident = misc_pool.tile([E, 2 * E], in_dt)
        nc.sync.dma_start(ident[:], id_in[:, :])
        part = misc_pool.tile([E, B_LOC], f32)

        # All input DMAs issued up front; big plane transfers first, spread
        # over three queues (two HWDGE rings + SWDGE) so they stream
        # concurrently; bmn rows are only needed at each row's end.
        mts, bts = [], []
        for r in range(B_LOC):
            sl = slots[r]
            q = (nc.gpsimd, nc.sync, nc.scalar)[r % 3]
            mt = m_pool.tile([E, K, sl], in_dt, tag=f"m{r}")
            q.dma_start(mt[:], m_in[r, :, :, 0:sl])
            mts.append(mt)
        for r in range(B_LOC):
            sl = slots[r]
            q = nc.sync if r % 2 == 1 else nc.scalar
            bt = b_pool.tile([E, sl], f16, tag=f"bmn{r}")
            q.dma_start(bt[:], bmn_in[r, :, 0:sl])
            bts.append(bt)

        for r in range(B_LOC):
            sl = slots[r]
            mt = mts[r]
            ps = psum_pool.tile([E, T], f32, tag="ps")
            chunks = [(c, min(CH, sl - c)) for c in range(0, sl, CH)]
            for c0, cw in chunks:
                if fp8:
                    lhsT = bass.AP(ident[:].tensor, 0, [[2 * E, E], [E, 2], [1, E]])
                    for kp in range(K // 2):
                        rhs = bass.AP(
                            mt[:].tensor, 2 * kp * sl + c0,
                            [[K * sl, E], [sl, 2], [1, cw]],
                        )
                        nc.tensor.matmul(
                            ps[:, c0:c0 + cw], lhsT=lhsT, rhs=rhs,
                            start=(kp == 0), stop=(kp == K // 2 - 1),
                            perf_mode=mybir.MatmulPerfMode.DoubleRow,
                        )
                else:
                    lhsT = bass.AP(ident[:].tensor, 0, [[2 * E, E], [1, E]])
                    for k in range(K):
                        rhs = bass.AP(
                            mt[:].tensor, k * sl + c0,
                            [[K * sl, E], [1, cw]],
                        )
                        nc.tensor.matmul(
                            ps[:, c0:c0 + cw], lhsT=lhsT, rhs=rhs,
                            start=(k == 0), stop=(k == K - 1),
                        )
            # Fused multiply+reduce on DVE: prod = ps * bmn (written fp16,
            # never re-read), accum_out = sum_s -> partials column.
            prod = prod_pool.tile([E, T], f16, tag="prod")
            nc.vector.scalar_tensor_tensor(
                prod[:, 0:sl], ps[:, 0:sl], 1.0, bts[r][:],
                op0=mybir.AluOpType.mult, op1=mybir.AluOpType.mult,
                accum_out=part[:, r:r + 1],
            )
        nc.scalar.dma_start(p_out[:, :], part[:])

    nc.compile()
    return nc


def kernel(base_emb, mapped_ctx, seq_lens, neg_ids):
    global LAST_RESULTS
    import ml_dtypes
    from concourse import bass_utils

    base = np.ascontiguousarray(np.asarray(base_emb, dtype=np.float32))
    mctx = np.asarray(mapped_ctx, dtype=np.float32)
    seq = np.asarray(seq_lens, dtype=np.int32)
    nids = np.asarray(neg_ids, dtype=np.int32)

    in_np_dt = ml_dtypes.float8_e4m3 if MODE == "pacc8" else np.float16

    # Host prep (sharding + per-batch-element negative gather per the
    # sharding hint; mask and per-step normalization fold into the linear
    # prefactors of mapped_ctx / base).
    neg_sum = base.reshape(B * T, E)[nids].sum(axis=1)             # [B, E]
    bmn = np.ascontiguousarray(
        ((base - neg_sum[:, None, :]) / SCALE).transpose(0, 2, 1)
        .astype(np.float16))                                       # [B, E, T]

    alpha = (SCALE * (T - 1.0) / (T - 1.0 - np.arange(K))).astype(np.float32)
    mask = (np.arange(T)[None, :] < seq[:, None]).astype(np.float32)  # [B, T]
    m_base = ((mctx.transpose(0, 2, 3, 1)                          # [B,E,K,T]
               * alpha[None, None, :, None]
               * mask[:, None, None, :]).astype(in_np_dt))
    mm = np.zeros((B, E, K, T), in_np_dt)
    for k in range(K):                                             # shift by k+1
        mm[:, :, k, k + 1:] = m_base[:, :, k, :T - 1 - k]

    ident = np.zeros((E, 2 * E), in_np_dt)
    ident[:, :E][np.arange(E), np.arange(E)] = 1.0
    ident[:, E:][np.arange(E), np.arange(E)] = 1.0

    # Row widths: all loss terms of row b live at s < seq_b + K + 1. Sort by
    # width, snake-assign to cores; slot width = max within the rank band.
    if TRIM:
        sl_r = np.minimum(seq + K + 1, T)
        sl128 = ((sl_r + 127) // 128) * 128
        order = np.argsort(-sl128, kind="stable")
        slots = tuple(int(sl128[order[j * NCORES]]) for j in range(B_LOC))
    else:
        order = np.arange(B, dtype=np.int64)
        slots = (T,) * B_LOC

    key = ("nc", MODE, slots)
    if key not in _CACHE:
        _CACHE[key] = _build(MODE, slots)
    nc = _CACHE[key]

    in_maps = []
    for c in range(NCORES):
        rows = [int(order[j * NCORES + c]) for j in range(B_LOC)]
        in_maps.append({
            "m8": np.ascontiguousarray(mm[rows]),
            "bmn": np.ascontiguousarray(bmn[rows]),
            "ident": ident,
        })

    res = bass_utils.run_bass_kernel_spmd(
        nc, in_maps, core_ids=list(range(NCORES)), trace=TRACE, **TRACE_KWARGS
    )
    LAST_RESULTS = res

    s_total = sum(float(r["P"].sum(dtype=np.float64)) for r in res.results)
    loss = -s_total / (K * B * (T - 1.0))
    return np.float32(loss)
